# revision 11
# baseline (speedup 1.0000x reference)
"""AltRobertaSelfAttention (relative_key_query) distributed Trainium2 kernel.

Sharding: 8 cores = 4 batches x 2 head-groups (8 heads each). Fully data
parallel; host slices inputs / concatenates outputs.

Per-core algorithm (batch b, heads hg*8..hg*8+7), all matmuls bf16:
  qT/kT = W^T @ hs^T   (d-major, for QK^T and bias matmuls)
  v     = hs @ Wv      (s-major, with an appended ones-column per head ->
                        PV matmul also produces the softmax denominator)
  Per head h:
    AD = q_h @ Erev^T, BD = k_h @ E^T   written (band only) to DRAM bf16
    scoresT[r,l] = k_r . q_l   (PSUM)
    T_B[r,l] = BD[r, 1023+l-r]  <- flat-stride skew read (contiguous rows)
    T_A[r,l] = AD[l, 1023-l+r]^T <- xbar dma-transpose of the skewed view
    probsT = exp((scoresT+T_A+T_B)/8 + mask[r])    (no max-sub; logits tiny)
    ctx[l, 0:64 | sum] = probsT^T @ [v_h | ones]; ctx /= sum
"""

import numpy as np
import ml_dtypes

import concourse.bass as bass
import concourse.mybir as mybir
from concourse.ap import AP
from concourse.bass_utils import run_bass_kernel_spmd

BF16 = mybir.dt.bfloat16
F32 = mybir.dt.float32
S = 1024
HID = 1024
HD = 64
NHEAD_CORE = 8
W2 = 2047  # dist-emb rows (2*MAXPOS-1)
NT = 8  # 128-row tiles per S
AF = mybir.ActivationFunctionType
ALU = mybir.AluOpType

_cache = {}


class Sched:
    """Collects per-engine instruction streams with cumulative sem values."""

    def __init__(self):
        self.streams = {k: [] for k in ("sync", "tensor", "scalar", "vector", "gpsimd")}
        self.cnt = {}  # sem name -> cumulative value
        self.hiwater = {}  # (stream, sem) -> last waited value

    def wait(self, stream, sem, val):
        if val <= 0:
            return
        key = (stream, sem)
        if self.hiwater.get(key, -1) >= val:
            return
        self.hiwater[key] = val
        self.streams[stream].append(("wait", sem, val))

    def op(self, stream, kind, inc=None, inc_by=1, **kw):
        self.streams[stream].append(("op", kind, kw, inc, inc_by))
        if inc is not None:
            self.cnt[inc] = self.cnt.get(inc, 0) + inc_by
            return self.cnt[inc]
        return None

    def val(self, sem):
        return self.cnt.get(sem, 0)


def _build():
    nc = bass.Bass()

    # ---------------- I/O ----------------
    hsT_ext = nc.dram_tensor("hsT", [HID, S], BF16, kind="ExternalInput")
    wq_ext = nc.dram_tensor("wq", [HID, 512], BF16, kind="ExternalInput")
    wk_ext = nc.dram_tensor("wk", [HID, 512], BF16, kind="ExternalInput")
    wv_ext = nc.dram_tensor("wv", [HID, 512], BF16, kind="ExternalInput")
    et_ext = nc.dram_tensor("et", [HD, 2048], BF16, kind="ExternalInput")
    etr_ext = nc.dram_tensor("etr", [HD, 2048], BF16, kind="ExternalInput")
    bq_ext = nc.dram_tensor("bq", [512], F32, kind="ExternalInput")
    bk_ext = nc.dram_tensor("bk", [512], F32, kind="ExternalInput")
    bv_ext = nc.dram_tensor("bv", [128, 512], F32, kind="ExternalInput")
    mask_ext = nc.dram_tensor("mask", [S], F32, kind="ExternalInput")
    out_ext = nc.dram_tensor("out", [S, 512], F32, kind="ExternalOutput")

    dbg = {}
    if _cache.get("debug"):
        dbg["qt"] = nc.dram_tensor("dbg_qt", [128, 4096], BF16, kind="ExternalOutput")
        dbg["kt"] = nc.dram_tensor("dbg_kt", [128, 4096], BF16, kind="ExternalOutput")
        dbg["v"] = nc.dram_tensor("dbg_v", [128, 4160], BF16, kind="ExternalOutput")
        dbg["probs"] = nc.dram_tensor("dbg_probs", [128, 8192], BF16, kind="ExternalOutput")
        dbg["ad"] = nc.dram_tensor("dbg_ad", [S * W2 + 64], BF16, kind="ExternalOutput")
        dbg["bd"] = nc.dram_tensor("dbg_bd", [S * W2 + 64], BF16, kind="ExternalOutput")
        dbg["sums"] = nc.dram_tensor("dbg_sums", [128, 4], F32, kind="ExternalOutput")

    # DRAM scratch (double buffered across heads)
    ad = [nc.dram_tensor(f"ad{i}", [S * W2 + 64], BF16) for i in range(2)]
    bd = [nc.dram_tensor(f"bd{i}", [S * W2 + 64], BF16) for i in range(2)]

    from contextlib import ExitStack
    with ExitStack() as stack:
        e = stack.enter_context
        hsT_sb = e(nc.sbuf_tensor([128, 8 * S], BF16))
        wq_sb = e(nc.sbuf_tensor([128, 8 * 512], BF16))
        wk_sb = e(nc.sbuf_tensor([128, 8 * 512], BF16))
        wv_sb = e(nc.sbuf_tensor([128, 8 * 512], BF16))
        qt_sb = e(nc.sbuf_tensor([128, 4 * S], BF16))
        kt_sb = e(nc.sbuf_tensor([128, 4 * S], BF16))
        v_sb = e(nc.sbuf_tensor([128, 8 * 520], BF16))
        et_sb = e(nc.sbuf_tensor([128, 2048], BF16))
        etr_sb = e(nc.sbuf_tensor([128, 2048], BF16))
        mask_sb = e(nc.sbuf_tensor([128, 8], F32))
        bq_sb = e(nc.sbuf_tensor([128, 4], F32))
        bk_sb = e(nc.sbuf_tensor([128, 4], F32))
        bv_sb = e(nc.sbuf_tensor([128, 512], F32))
        probs_sb = e(nc.sbuf_tensor([128, 8 * S], BF16))
        sc_sb = e(nc.sbuf_tensor([128, 2 * S], BF16))
        tb_sb = e(nc.sbuf_tensor([128, 2 * S], BF16))
        ta_sb = e(nc.sbuf_tensor([128, 2 * S], BF16))
        bev_sb = e(nc.sbuf_tensor([128, 4 * 1536], F32))
        ctx_sb = e(nc.sbuf_tensor([128, 2 * 512], F32))
        rc_sb = e(nc.sbuf_tensor([128, 16], F32))
        sum_sb = e(nc.sbuf_tensor([128, 16], F32))
        dbg_sums_sb = e(nc.sbuf_tensor([128, 4], F32))
        ps = e(nc.psum_tensor([128, 8 * 512], F32))
        ldin_sem = e(nc.semaphore("ldin"))
        wout_sem = e(nc.semaphore("wout"))
        rin_sem = e(nc.semaphore("rin"))
        outs_sem = e(nc.semaphore("outs"))
        pe_sem = e(nc.semaphore("pe"))
        act_sem = e(nc.semaphore("act"))
        dve_sem = e(nc.semaphore("dve"))
        gp_sem = e(nc.semaphore("gp"))
        block = e(nc.Block())
        sch = Sched()
        sems = dict(ldin=ldin_sem, wout=wout_sem, rin=rin_sem, outs=outs_sem,
                    pe=pe_sem, act=act_sem, dve=dve_sem, gp=gp_sem)

        def psb(i):  # psum bank i as [128, 512] AP
            return ps[:, i * 512:(i + 1) * 512]

        # ---------------- input loads (sync) ----------------
        sch.op("sync", "dma", inc="ldin", inc_by=16,
               out=AP(hsT_sb[:].tensor, 0, [[8 * S, 128], [S, 8], [1, S]]),
               in_=AP(hsT_ext[:].tensor, 0, [[S, 128], [128 * S, 8], [1, S]]))
        for w_sb, w_ext in ((wq_sb, wq_ext), (wk_sb, wk_ext), (wv_sb, wv_ext)):
            sch.op("sync", "dma", inc="ldin", inc_by=16,
                   out=AP(w_sb[:].tensor, 0, [[8 * 512, 128], [512, 8], [1, 512]]),
                   in_=AP(w_ext[:].tensor, 0, [[512, 128], [128 * 512, 8], [1, 512]]))
        for half in range(2):
            sch.op("sync", "dma", inc="ldin", inc_by=16,
                   out=et_sb[half * 64:half * 64 + 64, :], in_=et_ext[:])
            sch.op("sync", "dma", inc="ldin", inc_by=16,
                   out=etr_sb[half * 64:half * 64 + 64, :], in_=etr_ext[:])
        sch.op("sync", "dma", inc="ldin", inc_by=16,
               out=mask_sb[:], in_=AP(mask_ext[:].tensor, 0, [[1, 128], [128, 8]]))
        for b_sb, b_ext in ((bq_sb, bq_ext), (bk_sb, bk_ext)):
            sch.op("sync", "dma", inc="ldin", inc_by=16,
                   out=b_sb[:], in_=AP(b_ext[:].tensor, 0, [[1, 128], [128, 4]]))
        sch.op("sync", "dma", inc="ldin", inc_by=16,
               out=bv_sb[:], in_=bv_ext[:])
        ld_all = sch.val("ldin")

        # ones columns of v_aug (gpsimd memsets, early)
        for rt in range(NT):
            sch.op("gpsimd", "memset", inc="gp",
                   ap=AP(v_sb[:].tensor, rt * 520 + 64, [[8 * 520, 128], [65, 8]]),
                   const=1.0)
        gp_ones = sch.val("gp")

        # ---------------- phase A: projections ----------------
        # qT/kT: out[hd_chunk(128), s_half(512)] ; v: out[s_tile(128), d(512)]
        bank_evt = {}  # psum bank -> act/dve event that freed it

        def proj_qk(w_sb, t_sb, b_sb, name):
            for i in range(8):
                hdc, sh = i // 2, i % 2
                bank = i % 2
                ev = bank_evt.get(bank)
                if ev is not None:
                    sch.wait("tensor", ev[0], ev[1])
                sch.wait("tensor", "ldin", ld_all)
                for ci in range(8):
                    sch.op("tensor", "mm",
                           inc="pe" if ci == 7 else None,
                           out=psb(bank),
                           lhsT=w_sb[:, ci * 512 + 128 * hdc: ci * 512 + 128 * hdc + 128],
                           rhs=hsT_sb[:, ci * S + 512 * sh: ci * S + 512 * sh + 512],
                           start=ci == 0, stop=ci == 7)
                pe_v = sch.val("pe")
                sch.wait("scalar", "pe", pe_v)
                a_v = sch.op("scalar", "act",
                             inc="act",
                             out=t_sb[:, hdc * S + 512 * sh: hdc * S + 512 * sh + 512],
                             in_=psb(bank), func=AF.Identity,
                             bias=b_sb[:, hdc:hdc + 1], scale=1.0)
                bank_evt[bank] = ("act", a_v)

        proj_qk(wq_sb, qt_sb, bq_sb, "q")
        proj_qk(wk_sb, kt_sb, bk_sb, "k")
        for st in range(8):
            bank = st % 2
            ev = bank_evt.get(bank)
            if ev is not None:
                sch.wait("tensor", ev[0], ev[1])
            for ci in range(8):
                sch.op("tensor", "mm",
                       inc="pe" if ci == 7 else None,
                       out=psb(bank),
                       lhsT=hsT_sb[:, ci * S + 128 * st: ci * S + 128 * st + 128],
                       rhs=wv_sb[:, ci * 512: ci * 512 + 512],
                       start=ci == 0, stop=ci == 7)
            pe_v = sch.val("pe")
            sch.wait("vector", "pe", pe_v)
            d_v = sch.op("vector", "tt", inc="dve",
                         out=AP(v_sb[:].tensor, st * 520, [[8 * 520, 128], [65, 8], [1, 64]]),
                         in0=AP(ps[:].tensor, 0 + bank * 512, [[8 * 512, 128], [64, 8], [1, 64]]),
                         in1=AP(bv_sb[:].tensor, 0, [[512, 128], [64, 8], [1, 64]]),
                         op=ALU.add)
            bank_evt[bank] = ("dve", d_v)

        # ---------------- per-head pipeline ----------------
        # bias psum: banks 0,1 ; QK: banks 2,3 (rt even) 4,5 (rt odd); PV: 6,7
        bias_pp = [0]
        bev_free = [None] * 4      # wout event freeing bev buf
        scbuf_free = [None] * 2    # dve event (adds done) freeing sc buf
        tbbuf_free = [None] * 2    # dve event freeing tb/ta bufs
        qkbank_free = {}           # bank -> act event
        pvbank_free = {}           # bank -> dve event
        ctx_free = [None] * 2      # outs event freeing ctx buf
        head_read_done = [0, 0]    # rin value after reads of head h (parity)
        head_write_done = [0, 0]   # wout value after writes of head h (parity)
        pv_last_pe = 0             # pe value of last PV matmul (prev head)
        bev_i = [0]

        def qt_slice(t_sb, h, c0, w):
            hdc, po = h // 2, (h % 2) * 64
            t = t_sb[:, hdc * S + c0: hdc * S + c0 + w]
            return AP(t.tensor, t.offset + po * (4 * S), [[4 * S, 64], [1, w]])

        for h in range(NHEAD_CORE):
            par = h % 2
            AD, BD = ad[par], bd[par]

            # --- B1: bias matmuls -> DRAM band ---
            wout_start = sch.val("wout")
            for side, (src_sb, e_sb, dst) in enumerate(
                    (("A", etr_sb, AD), ("B", et_sb, BD))):
                src_t = qt_sb if side == 0 else kt_sb
                e_sb = etr_sb if side == 0 else et_sb
                for t in range(NT):
                    w0 = 896 - 128 * t
                    chunks = [1, 2, 3] if t < 4 else [0, 1, 2]
                    bi = bev_i[0] % 4
                    bev_i[0] += 1
                    if bev_free[bi] is not None:
                        sch.wait("scalar", "wout", bev_free[bi])
                    for ci, c in enumerate(chunks):
                        bank = bias_pp[0]
                        bias_pp[0] ^= 1
                        ev = bank_evt.get(bank)
                        if ev is not None:
                            sch.wait("tensor", ev[0], ev[1])
                        po = (h % 2) * 64
                        sch.op("tensor", "mm", inc="pe",
                               out=psb(bank),
                               lhsT=qt_slice(src_t, h, 128 * t, 128),
                               rhs=e_sb[po:po + 64, 512 * c: 512 * c + 512],
                               start=True, stop=True)
                        pe_v = sch.val("pe")
                        sch.wait("scalar", "pe", pe_v)
                        a_v = sch.op("scalar", "act", inc="act",
                                     out=bev_sb[:, bi * 1536 + 512 * ci: bi * 1536 + 512 * ci + 512],
                                     in_=psb(bank), func=AF.Identity, bias=0.0, scale=1.0)
                        bank_evt[bank] = ("act", a_v)
                    # DMA band write (gpsimd SWDGE, casts f32->bf16)
                    sch.wait("gpsimd", "act", sch.val("act"))
                    if h >= 2:
                        sch.wait("gpsimd", "rin", head_read_done[par])
                    off = w0 - 512 * chunks[0]
                    w_v = sch.op("gpsimd", "dma", inc="wout", inc_by=16,
                                 out=AP(dst[:].tensor, t * 128 * W2 + w0, [[W2, 128], [1, 1151]]),
                                 in_=bev_sb[:, bi * 1536 + off: bi * 1536 + off + 1151])
                    bev_free[bi] = w_v
            head_write_done[par] = sch.val("wout")

            # --- B2: QK + bias adds + exp, per r-tile ---
            for rt in range(NT):
                b0 = 2 + 2 * (rt % 2)
                sbuf_i = rt % 2
                # QK matmuls
                for half in range(2):
                    bank = b0 + half
                    ev = qkbank_free.get(bank)
                    if ev is not None:
                        sch.wait("tensor", "act", ev)
                    sch.op("tensor", "mm", inc="pe",
                           out=psb(bank),
                           lhsT=qt_slice(kt_sb, h, 128 * rt, 128),
                           rhs=qt_slice(qt_sb, h, 512 * half, 512),
                           start=True, stop=True)
                qk_pe = sch.val("pe")
                # skew reads (sync). T_B from BD, T_A via xbar from AD.
                sch.wait("sync", "wout", head_write_done[par])
                if tbbuf_free[sbuf_i] is not None:
                    sch.wait("sync", "dve", tbbuf_free[sbuf_i])
                sch.op("sync", "dma", inc="rin", inc_by=16,
                       out=tb_sb[:, sbuf_i * S: sbuf_i * S + S],
                       in_=AP(BD[:].tensor, 1023 + rt * 128 * 2046, [[2046, 128], [1, S]]))
                sch.op("sync", "dma", inc="rin", inc_by=16, transpose=True,
                       out=ta_sb[:, sbuf_i * S: sbuf_i * S + S],
                       in_=AP(AD[:].tensor, 1023 + rt * 128, [[2046, S], [1, 128]]))
                reads_v = sch.val("rin")
                # evict QK -> sc (ACT), then adds (DVE), then exp (ACT)
                sch.wait("scalar", "pe", qk_pe)
                if scbuf_free[sbuf_i] is not None:
                    sch.wait("scalar", "dve", scbuf_free[sbuf_i])
                for half in range(2):
                    a_v = sch.op("scalar", "act", inc="act",
                                 out=sc_sb[:, sbuf_i * S + 512 * half: sbuf_i * S + 512 * half + 512],
                                 in_=psb(b0 + half), func=AF.Identity, bias=0.0, scale=1.0)
                    qkbank_free[b0 + half] = a_v
                ev_evict = sch.val("act")
                sch.wait("vector", "act", ev_evict)
                sch.wait("vector", "rin", reads_v)
                sc = sc_sb[:, sbuf_i * S: sbuf_i * S + S]
                sch.op("vector", "tt", out=sc, in0=sc,
                       in1=tb_sb[:, sbuf_i * S: sbuf_i * S + S], op=ALU.add)
                d_v = sch.op("vector", "tt", inc="dve", out=sc, in0=sc,
                             in1=ta_sb[:, sbuf_i * S: sbuf_i * S + S], op=ALU.add)
                scbuf_free[sbuf_i] = d_v
                tbbuf_free[sbuf_i] = d_v
                sch.wait("scalar", "dve", d_v)
                sch.wait("scalar", "pe", pv_last_pe)  # probs WAR vs prev head PV
                sch.wait("scalar", "ldin", ld_all)
                sch.op("scalar", "act", inc="act",
                       out=probs_sb[:, rt * S: rt * S + S], in_=sc,
                       func=AF.Exp, bias=mask_sb[:, rt:rt + 1], scale=0.125)
            exps_done = sch.val("act")
            head_read_done[par] = sch.val("rin")

            # --- B3: PV per l-chunk; batched normalize; one store per head ---
            hp = h % 2
            if ctx_free[hp] is not None:
                sch.wait("vector", "outs", ctx_free[hp])
            for c in range(NT):
                bank = 6 + c % 2
                pv = ps[:, bank * 512: bank * 512 + 65]
                sch.wait("tensor", "act", exps_done)
                sch.wait("tensor", "gp", gp_ones)
                ev = pvbank_free.get(bank)
                if ev is not None:
                    sch.wait("tensor", "dve", ev)
                for rt in range(NT):
                    sch.op("tensor", "mm",
                           inc="pe" if rt == 7 else None,
                           out=pv,
                           lhsT=probs_sb[:, rt * S + 128 * c: rt * S + 128 * c + 128],
                           rhs=v_sb[:, rt * 520 + h * 65: rt * 520 + h * 65 + 65],
                           start=rt == 0, stop=rt == 7)
                pv_last_pe = sch.val("pe")
                sch.wait("vector", "pe", pv_last_pe)
                sch.op("vector", "ts", out=sum_sb[:, hp * 8 + c: hp * 8 + c + 1],
                       in0=ps[:, bank * 512 + 64: bank * 512 + 65],
                       scalar=1.0, op=ALU.mult)
                d_v = sch.op("vector", "copy", inc="dve",
                             out=ctx_sb[:, hp * 512 + c * 64: hp * 512 + c * 64 + 64],
                             in_=ps[:, bank * 512: bank * 512 + 64])
                pvbank_free[bank] = d_v
            # self-wait (same-engine RAW), then batched reciprocal
            sch.wait("vector", "dve", sch.val("dve"))
            sch.op("vector", "recip", inc="dve",
                   out=rc_sb[:, hp * 8: hp * 8 + 8], in_=sum_sb[:, hp * 8: hp * 8 + 8])
            sch.wait("vector", "dve", sch.val("dve"))
            for c in range(NT):
                sch.op("vector", "ts", inc="dve",
                       out=ctx_sb[:, hp * 512 + c * 64: hp * 512 + c * 64 + 64],
                       in0=ctx_sb[:, hp * 512 + c * 64: hp * 512 + c * 64 + 64],
                       scalar=rc_sb[:, hp * 8 + c: hp * 8 + c + 1], op=ALU.mult)
            d_v = sch.val("dve")
            sch.wait("gpsimd", "dve", d_v)
            o_v = sch.op("gpsimd", "dma", inc="outs", inc_by=16,
                         out=AP(out_ext[:].tensor, h * 64, [[512, 128], [512 * 128, 8], [1, 64]]),
                         in_=AP(ctx_sb[:].tensor, hp * 512, [[2 * 512, 128], [64, 8], [1, 64]]))
            ctx_free[hp] = o_v

        if dbg:
            for sem in ("pe", "act", "dve", "outs", "wout", "rin"):
                sch.wait("gpsimd", sem, sch.val(sem))
            sch.op("gpsimd", "dma", inc="outs", inc_by=16, out=dbg["qt"][:], in_=qt_sb[:])
            sch.op("gpsimd", "dma", inc="outs", inc_by=16, out=dbg["kt"][:], in_=kt_sb[:])
            sch.op("gpsimd", "dma", inc="outs", inc_by=16, out=dbg["v"][:], in_=v_sb[:])
            sch.op("gpsimd", "dma", inc="outs", inc_by=16, out=dbg["probs"][:], in_=probs_sb[:])
            sch.op("gpsimd", "dma", inc="outs", inc_by=16, out=dbg["ad"][:], in_=ad[1][:])
            sch.op("gpsimd", "dma", inc="outs", inc_by=16, out=dbg["bd"][:], in_=bd[1][:])
            sch.op("gpsimd", "dma", inc="outs", inc_by=16, out=dbg["sums"][:], in_=dbg_sums_sb[:])

        # ---------------- replay ----------------
        def replay(eng, stream):
            for item in sch.streams[stream]:
                if item[0] == "wait":
                    eng.wait_ge(sems[item[1]], item[2])
                    continue
                _, kind, kw, inc, inc_by = item
                if kind == "dma":
                    ins = eng.dma_start(out=kw["out"], in_=kw["in_"],
                                        transpose=kw.get("transpose", False))
                elif kind == "mm":
                    ins = eng.matmul(kw["out"], kw["lhsT"], kw["rhs"],
                                     start=kw["start"], stop=kw["stop"])
                elif kind == "act":
                    ins = eng.activation(kw["out"], kw["in_"], kw["func"],
                                         bias=kw["bias"], scale=kw["scale"])
                elif kind == "tt":
                    ins = eng.tensor_tensor(kw["out"], kw["in0"], kw["in1"], kw["op"])
                elif kind == "ts":
                    ins = eng.tensor_scalar(kw["out"], kw["in0"], kw["scalar"],
                                            None, kw["op"])
                elif kind == "recip":
                    ins = eng.reciprocal(kw["out"], kw["in_"])
                elif kind == "copy":
                    ins = eng.tensor_copy(kw["out"], kw["in_"])
                elif kind == "memset":
                    ins = eng.memset(kw["ap"], kw["const"])
                else:
                    raise ValueError(kind)
                if inc is not None:
                    ins.then_inc(sems[inc], inc_by)

        stack.enter_context(nc.allow_non_contiguous_dma(
            reason="tiny one-time transposed loads of mask/bias vectors"))

        @block.sync
        def _(eng):
            replay(eng, "sync")

        @block.tensor
        def _(eng):
            replay(eng, "tensor")

        @block.scalar
        def _(eng):
            replay(eng, "scalar")

        @block.vector
        def _(eng):
            replay(eng, "vector")

        @block.gpsimd
        def _(eng):
            replay(eng, "gpsimd")

    return nc


def kernel(hidden_states, attention_mask, Wq, bq, Wk, bk, Wv, bv, dist_emb):
    hidden_states = np.asarray(hidden_states, np.float32)
    attention_mask = np.asarray(attention_mask, np.float32)
    B = hidden_states.shape[0]
    bf = ml_dtypes.bfloat16

    if "nc" not in _cache:
        _cache["nc"] = _build()
    nc = _cache["nc"]

    E = np.asarray(dist_emb, np.float32)
    et = np.zeros((HD, 2048), np.float32)
    et[:, :W2] = E.T
    etr = np.zeros((HD, 2048), np.float32)
    etr[:, :W2] = E[::-1].T

    in_maps = []
    for core in range(8):
        b, hg = core // 2, core % 2
        cs = slice(hg * 512, (hg + 1) * 512)
        in_maps.append({
            "hsT": np.ascontiguousarray(hidden_states[b].T).astype(bf),
            "wq": np.ascontiguousarray(np.asarray(Wq, np.float32)[:, cs]).astype(bf),
            "wk": np.ascontiguousarray(np.asarray(Wk, np.float32)[:, cs]).astype(bf),
            "wv": np.ascontiguousarray(np.asarray(Wv, np.float32)[:, cs]).astype(bf),
            "et": et.astype(bf),
            "etr": etr.astype(bf),
            "bq": np.ascontiguousarray(np.asarray(bq, np.float32)[cs]),
            "bk": np.ascontiguousarray(np.asarray(bk, np.float32)[cs]),
            "bv": np.ascontiguousarray(np.broadcast_to(np.asarray(bv, np.float32)[cs], (128, 512))),
            "mask": np.ascontiguousarray(attention_mask[b, 0, 0, :], np.float32),
        })

    res = run_bass_kernel_spmd(nc, in_maps, core_ids=list(range(8)),
                               **_cache.get("run_kwargs", {}))
    out = np.zeros((B, S, 16 * HD), np.float32)
    for core in range(8):
        b, hg = core // 2, core % 2
        out[b, :, hg * 512:(hg + 1) * 512] = res.results[core]["out"]
    _cache["last_result"] = res
    return out


# revision 12
# speedup vs baseline: 1.1159x; 1.1159x over previous
"""AltRobertaSelfAttention (relative_key_query) distributed Trainium2 kernel.

Sharding: 8 cores = 4 batches x 2 head-groups (8 heads each). Fully data
parallel; host slices inputs / concatenates outputs.

Per-core algorithm (batch b, heads hg*8..hg*8+7), all matmuls bf16:
  qT/kT = W^T @ hs^T   (d-major, for QK^T and bias matmuls)
  v     = hs @ Wv      (s-major, with an appended ones-column per head ->
                        PV matmul also produces the softmax denominator)
  Per head h:
    AD = q_h @ Erev^T, BD = k_h @ E^T   written (band only) to DRAM bf16
    scoresT[r,l] = k_r . q_l   (PSUM)
    T_B[r,l] = BD[r, 1023+l-r]  <- flat-stride skew read (contiguous rows)
    T_A[r,l] = AD[l, 1023-l+r]^T <- xbar dma-transpose of the skewed view
    probsT = exp((scoresT+T_A+T_B)/8 + mask[r])    (no max-sub; logits tiny)
    ctxT[d|sum, l] = [v_h | ones]^T @ probsT       (unnormalized + sums row)
  Host divides by the sums row and transposes.

PE stream order per head: QK(h), bias-matmuls(h+1), PV(h) -- the next head's
bias work covers the DRAM round-trip latency of head h and keeps HAM warm.
"""

import numpy as np
import ml_dtypes

import concourse.bass as bass
import concourse.mybir as mybir
from concourse.ap import AP
from concourse.bass_utils import run_bass_kernel_spmd

BF16 = mybir.dt.bfloat16
F32 = mybir.dt.float32
S = 1024
HID = 1024
HD = 64
NHEAD_CORE = 8
W2 = 2047  # dist-emb rows (2*MAXPOS-1)
NT = 8  # 128-row tiles per S
WIN = 1152  # written band window per tile (covers the needed 1151)
AF = mybir.ActivationFunctionType
ALU = mybir.AluOpType

_cache = {}


class Sched:
    """Collects per-engine instruction streams with cumulative sem values."""

    def __init__(self):
        self.streams = {k: [] for k in ("sync", "tensor", "scalar", "vector", "gpsimd")}
        self.cnt = {}
        self.hiwater = {}

    def wait(self, stream, sem, val):
        if val is None or val <= 0:
            return
        key = (stream, sem)
        if self.hiwater.get(key, -1) >= val:
            return
        self.hiwater[key] = val
        self.streams[stream].append(("wait", sem, val))

    def op(self, stream, kind, inc=None, inc_by=1, **kw):
        self.streams[stream].append(("op", kind, kw, inc, inc_by))
        if inc is not None:
            self.cnt[inc] = self.cnt.get(inc, 0) + inc_by
            return self.cnt[inc]
        return None

    def val(self, sem):
        return self.cnt.get(sem, 0)


def _build():
    nc = bass.Bass()

    hsT_ext = nc.dram_tensor("hsT", [HID, S], BF16, kind="ExternalInput")
    wq_ext = nc.dram_tensor("wq", [HID, 512], BF16, kind="ExternalInput")
    wk_ext = nc.dram_tensor("wk", [HID, 512], BF16, kind="ExternalInput")
    wv_ext = nc.dram_tensor("wv", [HID, 512], BF16, kind="ExternalInput")
    et_ext = nc.dram_tensor("et", [HD, 2048], BF16, kind="ExternalInput")
    etr_ext = nc.dram_tensor("etr", [HD, 2048], BF16, kind="ExternalInput")
    bq_ext = nc.dram_tensor("bq", [512], F32, kind="ExternalInput")
    bk_ext = nc.dram_tensor("bk", [512], F32, kind="ExternalInput")
    bv_ext = nc.dram_tensor("bv", [128, 512], F32, kind="ExternalInput")
    mask_ext = nc.dram_tensor("mask", [S], F32, kind="ExternalInput")
    # per head: rows h*65..h*65+63 = unnormalized ctx^T, row h*65+64 = sums
    out_ext = nc.dram_tensor("out", [NHEAD_CORE * 65, S], F32, kind="ExternalOutput")

    ad = [nc.dram_tensor(f"ad{i}", [S * W2 + 256], BF16) for i in range(2)]
    bd = [nc.dram_tensor(f"bd{i}", [S * W2 + 256], BF16) for i in range(2)]

    from contextlib import ExitStack
    with ExitStack() as stack:
        e = stack.enter_context
        hsT_sb = e(nc.sbuf_tensor([128, 8 * S], BF16))
        wq_sb = e(nc.sbuf_tensor([128, 8 * 512], BF16))
        wk_sb = e(nc.sbuf_tensor([128, 8 * 512], BF16))
        wv_sb = e(nc.sbuf_tensor([128, 8 * 512], BF16))
        qt_sb = e(nc.sbuf_tensor([128, 4 * S], BF16))
        kt_sb = e(nc.sbuf_tensor([128, 4 * S], BF16))
        v_sb = e(nc.sbuf_tensor([128, 8 * 520], BF16))
        et_sb = e(nc.sbuf_tensor([128, 2048], BF16))
        etr_sb = e(nc.sbuf_tensor([128, 2048], BF16))
        mask_sb = e(nc.sbuf_tensor([128, 8], F32))
        bq_sb = e(nc.sbuf_tensor([128, 4], F32))
        bk_sb = e(nc.sbuf_tensor([128, 4], F32))
        bv_sb = e(nc.sbuf_tensor([128, 512], F32))
        probs_sb = e(nc.sbuf_tensor([128, 8 * S], BF16))
        sc_sb = e(nc.sbuf_tensor([128, 2 * S], BF16))
        tb_sb = e(nc.sbuf_tensor([128, 2 * S], BF16))
        ta_sb = e(nc.sbuf_tensor([128, 2 * S], BF16))
        bev_sb = e(nc.sbuf_tensor([128, 4 * WIN], F32))
        ctx_sb = e(nc.sbuf_tensor([65, 2 * S], F32))
        ps = e(nc.psum_tensor([128, 8 * 512], F32))
        ldin_sem = e(nc.semaphore("ldin"))
        wout_sem = e(nc.semaphore("wout"))
        rin_sem = e(nc.semaphore("rin"))
        outs_sem = e(nc.semaphore("outs"))
        pe_sem = e(nc.semaphore("pe"))
        act_sem = e(nc.semaphore("act"))
        dve_sem = e(nc.semaphore("dve"))
        gp_sem = e(nc.semaphore("gp"))
        block = e(nc.Block())
        sch = Sched()
        sems = dict(ldin=ldin_sem, wout=wout_sem, rin=rin_sem, outs=outs_sem,
                    pe=pe_sem, act=act_sem, dve=dve_sem, gp=gp_sem)

        def psb(i, w=512, nrow=128):
            return ps[:nrow, i * 512:i * 512 + w]

        # ---------------- input loads (sync) ----------------
        sch.op("sync", "dma", inc="ldin", inc_by=16,
               out=AP(hsT_sb[:].tensor, 0, [[8 * S, 128], [S, 8], [1, S]]),
               in_=AP(hsT_ext[:].tensor, 0, [[S, 128], [128 * S, 8], [1, S]]))
        for w_sb, w_ext in ((wq_sb, wq_ext), (wk_sb, wk_ext), (wv_sb, wv_ext)):
            sch.op("sync", "dma", inc="ldin", inc_by=16,
                   out=AP(w_sb[:].tensor, 0, [[8 * 512, 128], [512, 8], [1, 512]]),
                   in_=AP(w_ext[:].tensor, 0, [[512, 128], [128 * 512, 8], [1, 512]]))
        for half in range(2):
            sch.op("sync", "dma", inc="ldin", inc_by=16,
                   out=et_sb[half * 64:half * 64 + 64, :], in_=et_ext[:])
            sch.op("sync", "dma", inc="ldin", inc_by=16,
                   out=etr_sb[half * 64:half * 64 + 64, :], in_=etr_ext[:])
        sch.op("sync", "dma", inc="ldin", inc_by=16,
               out=mask_sb[:], in_=AP(mask_ext[:].tensor, 0, [[1, 128], [128, 8]]))
        for b_sb, b_ext in ((bq_sb, bq_ext), (bk_sb, bk_ext)):
            sch.op("sync", "dma", inc="ldin", inc_by=16,
                   out=b_sb[:], in_=AP(b_ext[:].tensor, 0, [[1, 128], [128, 4]]))
        sch.op("sync", "dma", inc="ldin", inc_by=16, out=bv_sb[:], in_=bv_ext[:])
        ld_all = sch.val("ldin")

        for rt in range(NT):
            sch.op("gpsimd", "memset", inc="gp",
                   ap=AP(v_sb[:].tensor, rt * 520 + 64, [[8 * 520, 128], [65, 8]]),
                   const=1.0)
        gp_ones = sch.val("gp")

        # ---------------- phase A: projections ----------------
        bank_evt = {}  # psum bank -> (sem, val) freeing it

        def wait_bank(b):
            ev = bank_evt.get(b)
            if ev is not None:
                sch.wait("tensor", ev[0], ev[1])

        def proj_qk(w_sb, t_sb, b_sb):
            for i in range(8):
                hdc, sh = i // 2, i % 2
                bank = i % 2
                wait_bank(bank)
                sch.wait("tensor", "ldin", ld_all)
                for ci in range(8):
                    sch.op("tensor", "mm", inc="pe" if ci == 7 else None,
                           out=psb(bank),
                           lhsT=w_sb[:, ci * 512 + 128 * hdc: ci * 512 + 128 * hdc + 128],
                           rhs=hsT_sb[:, ci * S + 512 * sh: ci * S + 512 * sh + 512],
                           start=ci == 0, stop=ci == 7)
                sch.wait("scalar", "pe", sch.val("pe"))
                a_v = sch.op("scalar", "act", inc="act",
                             out=t_sb[:, hdc * S + 512 * sh: hdc * S + 512 * sh + 512],
                             in_=psb(bank), func=AF.Identity,
                             bias=b_sb[:, hdc:hdc + 1], scale=1.0)
                bank_evt[bank] = ("act", a_v)

        proj_qk(wq_sb, qt_sb, bq_sb)
        proj_qk(wk_sb, kt_sb, bk_sb)
        for st in range(8):
            bank = st % 2
            wait_bank(bank)
            for ci in range(8):
                sch.op("tensor", "mm", inc="pe" if ci == 7 else None,
                       out=psb(bank),
                       lhsT=hsT_sb[:, ci * S + 128 * st: ci * S + 128 * st + 128],
                       rhs=wv_sb[:, ci * 512: ci * 512 + 512],
                       start=ci == 0, stop=ci == 7)
            sch.wait("vector", "pe", sch.val("pe"))
            d_v = sch.op("vector", "tt", inc="dve",
                         out=AP(v_sb[:].tensor, st * 520, [[8 * 520, 128], [65, 8], [1, 64]]),
                         in0=AP(ps[:].tensor, bank * 512, [[8 * 512, 128], [64, 8], [1, 64]]),
                         in1=AP(bv_sb[:].tensor, 0, [[512, 128], [64, 8], [1, 64]]),
                         op=ALU.add)
            bank_evt[bank] = ("dve", d_v)

        # ---------------- per-head pipeline ----------------
        # psum banks: bias groups [0,1,2] / [3,4,5] ping-pong; QK+PV share 6,7
        state = dict(bias_grp=0, bev_i=0, pv_last_pe=0)
        bev_free = [None] * 4
        scbuf_free = [None] * 2
        tbbuf_free = [None] * 2
        head_read_done = [0, 0]
        head_write_done = [0, 0]
        exps_done = [0] * NHEAD_CORE
        ctx_free = [None] * 2

        def qt_slice(t_sb, h, c0, w):
            hdc, po = h // 2, (h % 2) * 64
            t = t_sb[:, hdc * S + c0: hdc * S + c0 + w]
            return AP(t.tensor, t.offset + po * (4 * S), [[4 * S, 64], [1, w]])

        def emit_b1(h):
            par = h % 2
            for side in range(2):  # 0: AD (q, etr), 1: BD (k, et)
                src_t = qt_sb if side == 0 else kt_sb
                e_sb = etr_sb if side == 0 else et_sb
                dst = ad[par] if side == 0 else bd[par]
                po = (h % 2) * 64
                for t in range(NT):
                    w0 = 896 - 128 * t
                    chunks = [1, 2, 3] if t < 4 else [0, 1, 2]
                    grp = state["bias_grp"]
                    state["bias_grp"] ^= 1
                    banks = [grp * 3, grp * 3 + 1, grp * 3 + 2]
                    bi = state["bev_i"] % 4
                    state["bev_i"] += 1
                    use_act = (t % 8) < 5  # 10 of 16 tiles evict on ACT
                    ev_stream = "scalar" if use_act else "vector"
                    for ci, c in enumerate(chunks):
                        wait_bank(banks[ci])
                        sch.op("tensor", "mm", inc="pe" if ci == 2 else None,
                               out=psb(banks[ci]),
                               lhsT=qt_slice(src_t, h, 128 * t, 128),
                               rhs=e_sb[po:po + 64, 512 * c: 512 * c + 512],
                               start=True, stop=True)
                    pe_v = sch.val("pe")
                    sch.wait(ev_stream, "pe", pe_v)
                    if bev_free[bi] is not None:
                        sch.wait(ev_stream, "wout", bev_free[bi])
                    off = w0 - 512 * chunks[0]
                    src_ps = ps[:, banks[0] * 512 + off: banks[0] * 512 + off + WIN]
                    dst_bev = bev_sb[:, bi * WIN: bi * WIN + WIN]
                    if use_act:
                        ev = sch.op("scalar", "act", inc="act", out=dst_bev,
                                    in_=src_ps, func=AF.Identity, bias=0.0, scale=1.0)
                        sem_name = "act"
                    else:
                        ev = sch.op("vector", "copy", inc="dve", out=dst_bev, in_=src_ps)
                        sem_name = "dve"
                    for b in banks:
                        bank_evt[b] = (sem_name, ev)
                    sch.wait("gpsimd", sem_name, ev)
                    if h >= 2:
                        sch.wait("gpsimd", "rin", head_read_done[par])
                    w_v = sch.op("gpsimd", "dma", inc="wout", inc_by=16,
                                 out=AP(dst[:].tensor, t * 128 * W2 + w0, [[W2, 128], [1, WIN]]),
                                 in_=dst_bev)
                    bev_free[bi] = w_v
            head_write_done[par] = sch.val("wout")

        def emit_qk_softmax(h):
            par = h % 2
            for rt in range(NT):
                sbuf_i = rt % 2
                # skew reads for this r-tile (sync)
                sch.wait("sync", "wout", head_write_done[par])
                if tbbuf_free[sbuf_i] is not None:
                    sch.wait("sync", "dve", tbbuf_free[sbuf_i])
                sch.op("sync", "dma", inc="rin", inc_by=16,
                       out=tb_sb[:, sbuf_i * S: sbuf_i * S + S],
                       in_=AP(bd[par][:].tensor, 1023 + rt * 128 * 2046,
                              [[2046, 128], [1, S]]))
                sch.op("sync", "dma", inc="rin", inc_by=16, transpose=True,
                       out=ta_sb[:, sbuf_i * S: sbuf_i * S + S],
                       in_=AP(ad[par][:].tensor, 1023 + rt * 128, [[2046, S], [1, 128]]))
                reads_v = sch.val("rin")
                d_v = 0
                for half in range(2):
                    bank = 6 + (rt * 2 + half) % 2
                    wait_bank(bank)
                    sch.op("tensor", "mm", inc="pe",
                           out=psb(bank),
                           lhsT=qt_slice(kt_sb, h, 128 * rt, 128),
                           rhs=qt_slice(qt_sb, h, 512 * half, 512),
                           start=True, stop=True)
                    qk_pe = sch.val("pe")
                    sc = sc_sb[:, sbuf_i * S + 512 * half: sbuf_i * S + 512 * half + 512]
                    sch.wait("scalar", "pe", qk_pe)
                    if scbuf_free[sbuf_i] is not None:
                        sch.wait("scalar", "dve", scbuf_free[sbuf_i])
                    a_v = sch.op("scalar", "act", inc="act", out=sc,
                                 in_=psb(bank), func=AF.Identity, bias=0.0, scale=1.0)
                    bank_evt[bank] = ("act", a_v)
                    sch.wait("vector", "act", a_v)
                    sch.wait("vector", "rin", reads_v)
                    sch.op("vector", "tt", out=sc, in0=sc,
                           in1=tb_sb[:, sbuf_i * S + 512 * half: sbuf_i * S + 512 * half + 512],
                           op=ALU.add)
                    d_v = sch.op("vector", "tt", inc="dve", out=sc, in0=sc,
                                 in1=ta_sb[:, sbuf_i * S + 512 * half: sbuf_i * S + 512 * half + 512],
                                 op=ALU.add)
                    sch.wait("scalar", "dve", d_v)
                    sch.wait("scalar", "pe", state["pv_last_pe"])
                    sch.wait("scalar", "ldin", ld_all)
                    sch.op("scalar", "act", inc="act",
                           out=probs_sb[:, rt * S + 512 * half: rt * S + 512 * half + 512],
                           in_=sc, func=AF.Exp, bias=mask_sb[:, rt:rt + 1], scale=0.125)
                scbuf_free[sbuf_i] = d_v
                tbbuf_free[sbuf_i] = d_v
            exps_done[h] = sch.val("act")
            head_read_done[par] = sch.val("rin")

        def emit_pv(h):
            hp = h % 2
            if ctx_free[hp] is not None:
                sch.wait("vector", "outs", ctx_free[hp])
            for c in range(2):
                bank = 6 + c
                pv = psb(bank, 512, 65)
                sch.wait("tensor", "act", exps_done[h])
                sch.wait("tensor", "gp", gp_ones)
                wait_bank(bank)
                for rt in range(NT):
                    sch.op("tensor", "mm", inc="pe" if rt == 7 else None,
                           out=pv,
                           lhsT=v_sb[:, rt * 520 + h * 65: rt * 520 + h * 65 + 65],
                           rhs=probs_sb[:, rt * S + 512 * c: rt * S + 512 * c + 512],
                           start=rt == 0, stop=rt == 7)
                state["pv_last_pe"] = sch.val("pe")
                sch.wait("vector", "pe", state["pv_last_pe"])
                d_v = sch.op("vector", "copy", inc="dve",
                             out=ctx_sb[:, hp * S + 512 * c: hp * S + 512 * c + 512],
                             in_=pv)
                bank_evt[bank] = ("dve", d_v)
            sch.wait("gpsimd", "dve", sch.val("dve"))
            o_v = sch.op("gpsimd", "dma", inc="outs", inc_by=16,
                         out=out_ext[h * 65:(h + 1) * 65, :],
                         in_=ctx_sb[:, hp * S: hp * S + S])
            ctx_free[hp] = o_v

        emit_b1(0)
        for h in range(NHEAD_CORE):
            emit_qk_softmax(h)
            if h + 1 < NHEAD_CORE:
                emit_b1(h + 1)
            emit_pv(h)

        # ---------------- replay ----------------
        stack.enter_context(nc.allow_non_contiguous_dma(
            reason="tiny one-time transposed loads of mask/bias vectors"))

        def replay(eng, stream):
            for item in sch.streams[stream]:
                if item[0] == "wait":
                    eng.wait_ge(sems[item[1]], item[2])
                    continue
                _, kind, kw, inc, inc_by = item
                if kind == "dma":
                    ins = eng.dma_start(out=kw["out"], in_=kw["in_"],
                                        transpose=kw.get("transpose", False))
                elif kind == "mm":
                    ins = eng.matmul(kw["out"], kw["lhsT"], kw["rhs"],
                                     start=kw["start"], stop=kw["stop"])
                elif kind == "act":
                    ins = eng.activation(kw["out"], kw["in_"], kw["func"],
                                         bias=kw["bias"], scale=kw["scale"])
                elif kind == "tt":
                    ins = eng.tensor_tensor(kw["out"], kw["in0"], kw["in1"], kw["op"])
                elif kind == "copy":
                    ins = eng.tensor_copy(kw["out"], kw["in_"])
                elif kind == "memset":
                    ins = eng.memset(kw["ap"], kw["const"])
                else:
                    raise ValueError(kind)
                if inc is not None:
                    ins.then_inc(sems[inc], inc_by)

        @block.sync
        def _(eng):
            replay(eng, "sync")

        @block.tensor
        def _(eng):
            replay(eng, "tensor")

        @block.scalar
        def _(eng):
            replay(eng, "scalar")

        @block.vector
        def _(eng):
            replay(eng, "vector")

        @block.gpsimd
        def _(eng):
            replay(eng, "gpsimd")

    return nc


def kernel(hidden_states, attention_mask, Wq, bq, Wk, bk, Wv, bv, dist_emb):
    hidden_states = np.asarray(hidden_states, np.float32)
    attention_mask = np.asarray(attention_mask, np.float32)
    B = hidden_states.shape[0]
    bf = ml_dtypes.bfloat16

    if "nc" not in _cache:
        _cache["nc"] = _build()
    nc = _cache["nc"]

    E = np.asarray(dist_emb, np.float32)
    et = np.zeros((HD, 2048), np.float32)
    et[:, :W2] = E.T
    etr = np.zeros((HD, 2048), np.float32)
    etr[:, :W2] = E[::-1].T

    in_maps = []
    for core in range(8):
        b, hg = core // 2, core % 2
        cs = slice(hg * 512, (hg + 1) * 512)
        in_maps.append({
            "hsT": np.ascontiguousarray(hidden_states[b].T).astype(bf),
            "wq": np.ascontiguousarray(np.asarray(Wq, np.float32)[:, cs]).astype(bf),
            "wk": np.ascontiguousarray(np.asarray(Wk, np.float32)[:, cs]).astype(bf),
            "wv": np.ascontiguousarray(np.asarray(Wv, np.float32)[:, cs]).astype(bf),
            "et": et.astype(bf),
            "etr": etr.astype(bf),
            "bq": np.ascontiguousarray(np.asarray(bq, np.float32)[cs]),
            "bk": np.ascontiguousarray(np.asarray(bk, np.float32)[cs]),
            "bv": np.ascontiguousarray(np.broadcast_to(
                np.asarray(bv, np.float32)[cs], (128, 512))),
            "mask": np.ascontiguousarray(attention_mask[b, 0, 0, :], np.float32),
        })

    res = run_bass_kernel_spmd(nc, in_maps, core_ids=list(range(8)),
                               **_cache.get("run_kwargs", {}))
    out = np.zeros((B, S, 16 * HD), np.float32)
    for core in range(8):
        b, hg = core // 2, core % 2
        r = res.results[core]["out"]  # [520, 1024]
        for h in range(NHEAD_CORE):
            ctxT = r[h * 65: h * 65 + 64, :]       # [64, S] unnormalized
            sums = r[h * 65 + 64, :]               # [S]
            out[b, :, hg * 512 + h * 64: hg * 512 + (h + 1) * 64] = (ctxT / sums).T
    _cache["last_result"] = res
    return out
